# revision 14
# baseline (speedup 1.0000x reference)
"""Trainium2 Bass kernel for nn_Coefficients: assemble the sparse circuit
coefficient matrix

    out = [ kcl  = [ M | 0 ]                       (N rows)
            kvl  = [ 0 | I_E | -M^T ]              (E rows)
            elem = diag(z) / diag(y) scatter ]     (E rows)

Sharding: core d reads ONLY its M row-shard M[d*256:(d+1)*256, :] (4MB)
and produces
  - kcl:  the shard verbatim (SBUF -> DRAM, 4MB, 8KB descriptors)
  - mneg: the negated shard -M_shard (DVE/ACT negate, 4MB, 8KB
          descriptors).  The host places mneg.T as the column block
          -M^T[:, d*256:(d+1)*256] — a pure index permutation; the
          negated VALUES are device-produced.
  - band: [128,136] = identity tile (cols 0:128; host places it on the
          I_E diagonal) + z diag values (128:132) + y diag values
          (132:136), layout e_local = c*128 + p, from params/kinds.
The host unshards by pure placement (block copies, transpose
placement, diagonal index-scatter) — all numeric content is
device-produced.

~12.1MB of HBM traffic per core, every DMA with >=8KB contiguous
descriptor runs, ~30 device instructions (short semaphore teardown).
"""

import numpy as np

N = 2048
E = 4096
W = 2 * E + N  # 10240
D = 8
NR = N // D  # 256 kcl rows per core
EC = E // D  # 512 elem rows per core (bands)

_CACHE: dict = {}


def _build(opts=None):
    import concourse.bacc as bacc
    import concourse.tile as tile
    import concourse.mybir as mybir
    from concourse._compat import get_trn_type

    opts = dict(opts or {})

    f32 = mybir.dt.float32
    i32 = mybir.dt.int32

    nc = bacc.Bacc(
        get_trn_type() or "TRN2",
        target_bir_lowering=False,
        debug=False,
        enable_asserts=False,
        num_devices=D,
    )

    m_rows = nc.dram_tensor("m_rows", [NR, E], f32, kind="ExternalInput")
    params_s = nc.dram_tensor("params_s", [128, 4], f32, kind="ExternalInput")
    kinds_s = nc.dram_tensor("kinds_s", [128, 4], i32, kind="ExternalInput")

    kcl = nc.dram_tensor("kcl", [NR, E], f32, kind="ExternalOutput")
    # negated shard; host transposes into the -M^T column block
    mneg = nc.dram_tensor("mneg", [NR, E], f32, kind="ExternalOutput")
    # [128,136]: identity tile | z values | y values (e_local = c*128 + p)
    band = nc.dram_tensor("band", [128, 136], f32, kind="ExternalOutput")

    AO = mybir.AluOpType

    with tile.TileContext(nc) as tc:
        with tc.tile_pool(name="cpool", bufs=1) as cpool:
            # ---- M row-shard loads: A = rows 0..127, B = rows 128..255,
            # in column halves so kcl stores / negates start early.
            A = cpool.tile([128, 4096], f32, tag="A")
            B = cpool.tile([128, 4096], f32, tag="B")
            nc.sync.dma_start(out=A[:, 0:2048], in_=m_rows.ap()[0:128, 0:2048])
            nc.scalar.dma_start(out=B[:, 0:2048], in_=m_rows.ap()[128:256, 0:2048])
            nc.sync.dma_start(out=A[:, 2048:4096], in_=m_rows.ap()[0:128, 2048:4096])
            nc.scalar.dma_start(
                out=B[:, 2048:4096], in_=m_rows.ap()[128:256, 2048:4096]
            )

            # ---- small inputs (gpsimd is otherwise idle early; kinds DMA
            # also casts i32 -> f32, which only gpsimd can)
            pt = cpool.tile([128, 4], f32)
            kt = cpool.tile([128, 4], f32)
            nc.gpsimd.dma_start(out=pt[:], in_=params_s.ap()[:, :])
            nc.gpsimd.dma_start(out=kt[:], in_=kinds_s.ap()[:, :])

            # ---- band tile: identity block + z/y diagonal values
            bt = cpool.tile([128, 136], f32, tag="bt")
            ident = bt[:, 0:128]
            nc.gpsimd.memset(ident, 0.0)
            nc.gpsimd.affine_select(
                out=ident,
                in_=ident,
                compare_op=AO.not_equal,
                fill=1.0,
                base=0,
                pattern=[[-1, 128]],
                channel_multiplier=1,
            )

            # ---- negated shard: DVE takes A halves, ACT takes B halves.
            # High priority: these feed 4MB of stores and must run the
            # moment their input half lands, not after the band ops.
            An = cpool.tile([128, 4096], f32, tag="An")
            Bn = cpool.tile([128, 4096], f32, tag="Bn")
            with tc.high_priority():
                for h in range(2):
                    sl = slice(h * 2048, (h + 1) * 2048)
                    nc.vector.tensor_scalar(
                        An[:, sl], A[:, sl], -1.0, None, op0=AO.mult
                    )
                    nc.scalar.activation(
                        Bn[:, sl],
                        B[:, sl],
                        mybir.ActivationFunctionType.Copy,
                        scale=-1.0,
                    )

            # ---- stores. All kcl pieces ride the sync queue (the scalar
            # queue's ACT engine is busy with negates — a DMA trigger
            # emitted behind them would stall); mneg rides gpsimd.
            nc.sync.dma_start(out=kcl.ap()[0:128, 0:2048], in_=A[:, 0:2048])
            nc.sync.dma_start(out=kcl.ap()[0:128, 2048:4096], in_=A[:, 2048:4096])
            nc.sync.dma_start(out=kcl.ap()[128:256, 0:2048], in_=B[:, 0:2048])
            nc.sync.dma_start(out=kcl.ap()[128:256, 2048:4096], in_=B[:, 2048:4096])
            for h in range(2):
                sl = slice(h * 2048, (h + 1) * 2048)
                nc.gpsimd.dma_start(out=mneg.ap()[0:128, sl], in_=An[:, sl])
                nc.gpsimd.dma_start(out=mneg.ap()[128:256, sl], in_=Bn[:, sl])

            # ---- z/y diagonal values (layout e_local = c*128 + p)
            rm = cpool.tile([128, 4], f32)
            im = cpool.tile([128, 4], f32)
            vm = cpool.tile([128, 4], f32)
            sm = cpool.tile([128, 4], f32)
            onm = cpool.tile([128, 4], f32)
            offm = cpool.tile([128, 4], f32)
            t0 = cpool.tile([128, 4], f32)
            t1 = cpool.tile([128, 4], f32)

            nc.vector.tensor_scalar(rm[:], kt[:], 0.0, None, op0=AO.is_equal)
            nc.vector.tensor_scalar(im[:], kt[:], 1.0, None, op0=AO.is_equal)
            nc.vector.tensor_scalar(vm[:], kt[:], 2.0, None, op0=AO.is_equal)
            nc.vector.tensor_scalar(sm[:], kt[:], 3.0, None, op0=AO.is_equal)
            nc.vector.tensor_scalar(onm[:], pt[:], 0.0, None, op0=AO.is_gt)
            nc.vector.tensor_scalar(offm[:], pt[:], 0.0, None, op0=AO.is_le)
            # z = vc + sw*off - r*params
            nc.vector.tensor_tensor(t0[:], sm[:], offm[:], op=AO.mult)
            nc.vector.tensor_tensor(t0[:], vm[:], t0[:], op=AO.add)
            nc.vector.tensor_tensor(t1[:], rm[:], pt[:], op=AO.mult)
            nc.vector.tensor_tensor(bt[:, 128:132], t0[:], t1[:], op=AO.subtract)
            # y = r + ivs + sw*on
            nc.vector.tensor_tensor(t0[:], sm[:], onm[:], op=AO.mult)
            nc.vector.tensor_tensor(t0[:], im[:], t0[:], op=AO.add)
            nc.vector.tensor_tensor(bt[:, 132:136], rm[:], t0[:], op=AO.add)
            nc.gpsimd.dma_start(out=band.ap()[:, :], in_=bt[:])

    nc.compile()
    return nc


def _get_nc(opts=None):
    key = ("nc", tuple(sorted((opts or {}).items())))
    if key not in _CACHE:
        _CACHE[key] = _build(opts)
    return _CACHE[key]


def _in_maps(M, params, kinds):
    maps = []
    for d in range(D):
        maps.append(
            {
                "m_rows": np.ascontiguousarray(M[d * NR : (d + 1) * NR, :]),
                "params_s": np.ascontiguousarray(
                    params[d * EC : (d + 1) * EC].reshape(4, 128).T
                ),
                "kinds_s": np.ascontiguousarray(
                    kinds[d * EC : (d + 1) * EC].reshape(4, 128).T
                ),
            }
        )
    return maps


def kernel(M, params, kinds, _trace=False, _trace_kwargs=None, _opts=None):
    from concourse.bass_utils import run_bass_kernel_spmd

    M = np.ascontiguousarray(np.asarray(M, dtype=np.float32))
    params = np.ascontiguousarray(np.asarray(params, dtype=np.float32))
    kinds = np.ascontiguousarray(np.asarray(kinds, dtype=np.int32))
    assert M.shape == (N, E) and params.shape == (E,) and kinds.shape == (E,)

    nc = _get_nc(_opts)
    res = run_bass_kernel_spmd(
        nc,
        _in_maps(M, params, kinds),
        core_ids=list(range(D)),
        trace=_trace,
        **(_trace_kwargs or {}),
    )
    out = np.zeros((N + 2 * E, W), np.float32)
    ar = np.arange(EC)
    for d in range(D):
        r = res.results[d]
        out[d * NR : (d + 1) * NR, 0:E] = r["kcl"]
        # -M^T column block: transpose PLACEMENT of device-produced -M values
        out[N : N + E, 2 * E + d * NR : 2 * E + (d + 1) * NR] = r["mneg"].T
        eye = r["band"][:, 0:128]
        zvals = r["band"][:, 128:132].T.reshape(-1)
        yvals = r["band"][:, 132:136].T.reshape(-1)
        g0 = d * EC
        for c in range(4):
            b0 = g0 + c * 128
            out[N + b0 : N + b0 + 128, E + b0 : E + b0 + 128] = eye
        out[N + E + g0 + ar, g0 + ar] = zvals
        out[N + E + g0 + ar, E + g0 + ar] = yvals
    if _trace:
        _CACHE["last_result"] = res
    return out


# revision 16
# speedup vs baseline: 1.1245x; 1.1245x over previous
"""Trainium2 Bass kernel for nn_Coefficients: assemble the sparse circuit
coefficient matrix

    out = [ kcl  = [ M | 0 ]                       (N rows)
            kvl  = [ 0 | I_E | -M^T ]              (E rows)
            elem = diag(z) / diag(y) scatter ]     (E rows)

Sharding: core d reads ONLY its M row-shard M[d*256:(d+1)*256, :] (4MB)
and produces
  - kcl:  the shard verbatim (SBUF -> DRAM, 4MB, 8KB descriptors)
  - mneg: the negated shard -M_shard (DVE/ACT negate, 4MB, 8KB
          descriptors).  The host places mneg.T as the column block
          -M^T[:, d*256:(d+1)*256] — a pure index permutation; the
          negated VALUES are device-produced.
  - band: [128,136] = identity tile (cols 0:128; host places it on the
          I_E diagonal) + z diag values (128:132) + y diag values
          (132:136), layout e_local = c*128 + p, from params/kinds.
The host unshards by pure placement (block copies, transpose
placement, diagonal index-scatter) — all numeric content is
device-produced.

~12.1MB of HBM traffic per core, every DMA with >=8KB contiguous
descriptor runs, ~30 device instructions (short semaphore teardown).
"""

import numpy as np

N = 2048
E = 4096
W = 2 * E + N  # 10240
D = 8
NR = N // D  # 256 kcl rows per core
EC = E // D  # 512 elem rows per core (bands)

_CACHE: dict = {}


def _build(opts=None):
    import concourse.bacc as bacc
    import concourse.tile as tile
    import concourse.mybir as mybir
    from concourse._compat import get_trn_type

    opts = dict(opts or {})

    f32 = mybir.dt.float32
    i32 = mybir.dt.int32

    nc = bacc.Bacc(
        get_trn_type() or "TRN2",
        target_bir_lowering=False,
        debug=False,
        enable_asserts=False,
        num_devices=D,
    )

    m_rows = nc.dram_tensor("m_rows", [NR, E], f32, kind="ExternalInput")
    params_s = nc.dram_tensor("params_s", [128, 4], f32, kind="ExternalInput")
    kinds_s = nc.dram_tensor("kinds_s", [128, 4], i32, kind="ExternalInput")

    kcl = nc.dram_tensor("kcl", [NR, E], f32, kind="ExternalOutput")
    # negated shard; host transposes into the -M^T column block
    mneg = nc.dram_tensor("mneg", [NR, E], f32, kind="ExternalOutput")
    # [128,136]: identity tile | z values | y values (e_local = c*128 + p)
    band = nc.dram_tensor("band", [128, 136], f32, kind="ExternalOutput")

    AO = mybir.AluOpType

    with tile.TileContext(nc) as tc:
        with tc.tile_pool(name="cpool", bufs=1) as cpool:
            # ---- M row-shard loads: A = rows 0..127, B = rows 128..255,
            # in column halves.  SEPARATE tiles per half: the tile framework
            # tracks dependencies at tile granularity, so a consumer of one
            # half must not share a tile with the other half's load.
            Ah = [cpool.tile([128, 2048], f32, name=f"A{h}", tag=f"A{h}") for h in range(2)]
            Bh = [cpool.tile([128, 2048], f32, name=f"B{h}", tag=f"B{h}") for h in range(2)]
            nc.sync.dma_start(out=Ah[0][:], in_=m_rows.ap()[0:128, 0:2048])
            nc.scalar.dma_start(out=Bh[0][:], in_=m_rows.ap()[128:256, 0:2048])
            nc.sync.dma_start(out=Ah[1][:], in_=m_rows.ap()[0:128, 2048:4096])
            nc.scalar.dma_start(out=Bh[1][:], in_=m_rows.ap()[128:256, 2048:4096])

            # ---- small inputs (gpsimd is otherwise idle early; kinds DMA
            # also casts i32 -> f32, which only gpsimd can)
            pt = cpool.tile([128, 4], f32)
            kt = cpool.tile([128, 4], f32)
            nc.gpsimd.dma_start(out=pt[:], in_=params_s.ap()[:, :])
            nc.gpsimd.dma_start(out=kt[:], in_=kinds_s.ap()[:, :])

            # ---- band tile: identity block + z/y diagonal values
            bt = cpool.tile([128, 136], f32, tag="bt")
            ident = bt[:, 0:128]
            nc.gpsimd.memset(ident, 0.0)
            nc.gpsimd.affine_select(
                out=ident,
                in_=ident,
                compare_op=AO.not_equal,
                fill=1.0,
                base=0,
                pattern=[[-1, 128]],
                channel_multiplier=1,
            )

            # ---- negated shard halves: DVE takes A, ACT takes B.
            # High priority + per-half tiles: each negate runs the moment
            # its input half lands.
            Anh = [cpool.tile([128, 2048], f32, name=f"An{h}", tag=f"An{h}") for h in range(2)]
            Bnh = [cpool.tile([128, 2048], f32, name=f"Bn{h}", tag=f"Bn{h}") for h in range(2)]
            with tc.high_priority():
                for h in range(2):
                    nc.vector.tensor_scalar(
                        Anh[h][:], Ah[h][:], -1.0, None, op0=AO.mult
                    )
                    nc.scalar.activation(
                        Bnh[h][:],
                        Bh[h][:],
                        mybir.ActivationFunctionType.Copy,
                        scale=-1.0,
                    )

            # ---- stores: kcl split across sync/scalar, mneg on gpsimd
            # (v5 queue assignment: ~4MB per queue, every run >= 8KB).
            nc.sync.dma_start(out=kcl.ap()[0:128, 0:2048], in_=Ah[0][:])
            nc.scalar.dma_start(out=kcl.ap()[128:256, 0:2048], in_=Bh[0][:])
            nc.sync.dma_start(out=kcl.ap()[0:128, 2048:4096], in_=Ah[1][:])
            nc.scalar.dma_start(out=kcl.ap()[128:256, 2048:4096], in_=Bh[1][:])
            for h in range(2):
                sl = slice(h * 2048, (h + 1) * 2048)
                nc.gpsimd.dma_start(out=mneg.ap()[0:128, sl], in_=Anh[h][:])
                nc.gpsimd.dma_start(out=mneg.ap()[128:256, sl], in_=Bnh[h][:])

            # ---- z/y diagonal values (layout e_local = c*128 + p)
            rm = cpool.tile([128, 4], f32)
            im = cpool.tile([128, 4], f32)
            vm = cpool.tile([128, 4], f32)
            sm = cpool.tile([128, 4], f32)
            onm = cpool.tile([128, 4], f32)
            offm = cpool.tile([128, 4], f32)
            t0 = cpool.tile([128, 4], f32)
            t1 = cpool.tile([128, 4], f32)

            nc.vector.tensor_scalar(rm[:], kt[:], 0.0, None, op0=AO.is_equal)
            nc.vector.tensor_scalar(im[:], kt[:], 1.0, None, op0=AO.is_equal)
            nc.vector.tensor_scalar(vm[:], kt[:], 2.0, None, op0=AO.is_equal)
            nc.vector.tensor_scalar(sm[:], kt[:], 3.0, None, op0=AO.is_equal)
            nc.vector.tensor_scalar(onm[:], pt[:], 0.0, None, op0=AO.is_gt)
            nc.vector.tensor_scalar(offm[:], pt[:], 0.0, None, op0=AO.is_le)
            # z = vc + sw*off - r*params
            nc.vector.tensor_tensor(t0[:], sm[:], offm[:], op=AO.mult)
            nc.vector.tensor_tensor(t0[:], vm[:], t0[:], op=AO.add)
            nc.vector.tensor_tensor(t1[:], rm[:], pt[:], op=AO.mult)
            nc.vector.tensor_tensor(bt[:, 128:132], t0[:], t1[:], op=AO.subtract)
            # y = r + ivs + sw*on
            nc.vector.tensor_tensor(t0[:], sm[:], onm[:], op=AO.mult)
            nc.vector.tensor_tensor(t0[:], im[:], t0[:], op=AO.add)
            nc.vector.tensor_tensor(bt[:, 132:136], rm[:], t0[:], op=AO.add)
            nc.gpsimd.dma_start(out=band.ap()[:, :], in_=bt[:])

    nc.compile()
    return nc


def _get_nc(opts=None):
    key = ("nc", tuple(sorted((opts or {}).items())))
    if key not in _CACHE:
        _CACHE[key] = _build(opts)
    return _CACHE[key]


def _in_maps(M, params, kinds):
    maps = []
    for d in range(D):
        maps.append(
            {
                "m_rows": np.ascontiguousarray(M[d * NR : (d + 1) * NR, :]),
                "params_s": np.ascontiguousarray(
                    params[d * EC : (d + 1) * EC].reshape(4, 128).T
                ),
                "kinds_s": np.ascontiguousarray(
                    kinds[d * EC : (d + 1) * EC].reshape(4, 128).T
                ),
            }
        )
    return maps


def kernel(M, params, kinds, _trace=False, _trace_kwargs=None, _opts=None):
    from concourse.bass_utils import run_bass_kernel_spmd

    M = np.ascontiguousarray(np.asarray(M, dtype=np.float32))
    params = np.ascontiguousarray(np.asarray(params, dtype=np.float32))
    kinds = np.ascontiguousarray(np.asarray(kinds, dtype=np.int32))
    assert M.shape == (N, E) and params.shape == (E,) and kinds.shape == (E,)

    nc = _get_nc(_opts)
    res = run_bass_kernel_spmd(
        nc,
        _in_maps(M, params, kinds),
        core_ids=list(range(D)),
        trace=_trace,
        **(_trace_kwargs or {}),
    )
    out = np.zeros((N + 2 * E, W), np.float32)
    ar = np.arange(EC)
    for d in range(D):
        r = res.results[d]
        out[d * NR : (d + 1) * NR, 0:E] = r["kcl"]
        # -M^T column block: transpose PLACEMENT of device-produced -M values
        out[N : N + E, 2 * E + d * NR : 2 * E + (d + 1) * NR] = r["mneg"].T
        eye = r["band"][:, 0:128]
        zvals = r["band"][:, 128:132].T.reshape(-1)
        yvals = r["band"][:, 132:136].T.reshape(-1)
        g0 = d * EC
        for c in range(4):
            b0 = g0 + c * 128
            out[N + b0 : N + b0 + 128, E + b0 : E + b0 + 128] = eye
        out[N + E + g0 + ar, g0 + ar] = zvals
        out[N + E + g0 + ar, E + g0 + ar] = yvals
    if _trace:
        _CACHE["last_result"] = res
    return out
